# revision 37
# baseline (speedup 1.0000x reference)
"""EulerAttention Trainium2 kernel (v5: AV reassociated through x).

Per-core sharding: core c in 0..7 -> (batch b = c // 4, query block qb = c % 4,
1024 queries each).  Each core computes K features for its whole batch, Q
features for its query block, then flash-style scores/softmax and the output.

Key ideas:
- Output reassociation: out = softmax(sim) @ (x @ Wv.T) is computed as
  (softmax(sim) @ x) @ Wv.T.  U = E^T x contracts over keys (no per-batch
  duplication across cores) and the final Wv projection touches only this
  core's 1024 queries, so the whole duplicated-per-batch V projection
  (x @ Wv.T for all 4096 keys, f32r) disappears.  E^T x runs bf16 from a
  host-provided bf16 copy of x.
- Scores/rowsum/U are invariant under key permutation, so the host hands each
  core its batch x with the key blocks rotated to put the core's own query
  rows first.  Q features then reuse the same x loads as key blocks 0-1 and
  the whole kernel runs as ONE software-pipelined loop over key blocks.
- Q/K projections f32r for e-tiles 0..FP8_FROM-1 (phase-precision-critical,
  the 1/wavelength scale amplifies projection error), fp8 DoubleRow for the
  rest.  Q/K features (cos/sin theta) are stored fp8 and the [S,S] score
  matmuls run fp8 DoubleRow.
- K-side features are mean-centered per feature (host-computed E[cos theta_k]
  from the weights); the dropped cross terms are per-query constants that
  cancel in softmax normalization.
- exp(scores) and the U accumulator are bf16; K features stay SBUF-resident
  in rotating per-block slots (no DRAM roundtrips).
- Feature-map range reduction (turns-space magic round + add_range_wrap into
  the ACT Sin table) is spread across DVE and Pool (gpsimd).

kernel(**inputs) takes the full unsharded inputs from reference.setup_inputs()
and returns the full [B, S, D] output.
"""
import sys, math

sys.path.insert(0, "/opt/trn_rl_repo")

import numpy as np
import ml_dtypes

B, S, D = 2, 4096, 1024
NCORES = 8
QBLK = S // 4          # queries per core
ET = D // 128          # number of 128-row e/d tiles (8)
NSLOT = 2 * ET         # fp8 feature slots (cos/sin per et)
MAGIC = float(1.5 * 2**23)
TWOPI = 2.0 * math.pi
INV_SQRT_D = 1.0 / math.sqrt(D)
F8NP = ml_dtypes.float8_e4m3
BF16NP = ml_dtypes.bfloat16
FP8_FROM = 2           # e-tiles >= this run the Q/K projection in fp8 DoubleRow

_cache = {}


def _build_program(trace_sim=False):
    import concourse.bass as bass
    from concourse import bacc
    import concourse.mybir as mybir
    import concourse.tile as tile
    from contextlib import ExitStack

    f32 = mybir.dt.float32
    f32r = mybir.dt.float32r
    bf16 = mybir.dt.bfloat16
    f8 = mybir.dt.float8e4
    Act = mybir.ActivationFunctionType
    Alu = mybir.AluOpType
    PM = mybir.MatmulPerfMode

    s_keys, s_q = S, QBLK
    n_sblk = s_keys // 512       # 8 key blocks (block 0-1 = this core's queries)
    n_tt = s_keys // 128         # 32 key tiles
    n_qsb = s_q // 512           # 2 query blocks
    NS = s_q                     # resident query width (1024)
    n_ns = NS // 512             # N-splits for matmuls over queries

    nc = bacc.Bacc("TRN2", target_bir_lowering=False, debug=False)

    xT = nc.dram_tensor("xT", [D, s_keys], f32r, kind="ExternalInput").ap()
    XT8 = nc.dram_tensor("XT8", [D, s_keys], f8, kind="ExternalInput").ap()
    XB16 = nc.dram_tensor("XB16", [s_keys, D], bf16, kind="ExternalInput").ap()
    # f32r weight slices for the low e-tiles; fp8 full weights for DoubleRow
    WqT = nc.dram_tensor("WqT", [D, FP8_FROM * 128], f32r, kind="ExternalInput").ap()
    WkT = nc.dram_tensor("WkT", [D, FP8_FROM * 128], f32r, kind="ExternalInput").ap()
    W8C = D - FP8_FROM * 128   # fp8 weight columns (e-tiles >= FP8_FROM only)
    WQ8 = nc.dram_tensor("WQ8", [D, W8C], f8, kind="ExternalInput").ap()
    WK8 = nc.dram_tensor("WK8", [D, W8C], f8, kind="ExternalInput").ap()
    WvT = nc.dram_tensor("WvT", [D, D], bf16, kind="ExternalInput").ap()
    # packed per-partition constants: columns = (sc2 | bq2 | bk2 | bv | nac | nas) x ET
    CON = nc.dram_tensor("CON", [128, 6 * ET], f32, kind="ExternalInput").ap()

    OT = nc.dram_tensor("OT", [D, s_q], f32, kind="ExternalOutput").ap()

    with tile.TileContext(nc, trace_sim=trace_sim) as tc, ExitStack() as top:
        # ---- constants ----
        cpool = top.enter_context(tc.tile_pool(name="consts", bufs=1))
        ctile = cpool.tile([128, 6 * ET], f32, tag="ctile")
        nc.sync.dma_start(ctile[:], CON[:])
        sc2 = [ctile[:, i : i + 1] for i in range(ET)]
        bq2 = [ctile[:, ET + i : ET + i + 1] for i in range(ET)]
        bk2 = [ctile[:, 2 * ET + i : 2 * ET + i + 1] for i in range(ET)]
        bvt = [ctile[:, 3 * ET + i : 3 * ET + i + 1] for i in range(ET)]
        nac = [ctile[:, 4 * ET + i : 4 * ET + i + 1] for i in range(ET)]
        nas = [ctile[:, 5 * ET + i : 5 * ET + i + 1] for i in range(ET)]
        ones_col = cpool.tile([128, 2], bf16, tag="ones_col")
        nc.vector.memset(ones_col[:], 1.0)
        ones_row = cpool.tile([1, 128], bf16, tag="ones_row")
        nc.vector.memset(ones_row[:], 1.0)

        psum = top.enter_context(tc.tile_pool(name="psum", bufs=1, space="PSUM"))

        # ---- U = E^T x accumulator (bf16), [xd-tile][128 xd, queries] ----
        uacc = top.enter_context(tc.tile_pool(name="uacc", bufs=1))
        u_ac = [uacc.tile([128, NS], bf16, tag=f"u{xt}", name=f"uac{xt}")
                for xt in range(ET)]

        # ---- resident Q features + rotating K-feature / x-bf16 slots ----
        qres = top.enter_context(tc.tile_pool(name="qres", bufs=1))
        qa8 = qres.tile([128, NSLOT, NS], f8, tag="qa8")
        kpool = top.enter_context(tc.tile_pool(name="kres", bufs=2))
        xbpool = top.enter_context(tc.tile_pool(name="xbres", bufs=1))
        xbres = [xbpool.tile([128, 4, D], bf16, tag=f"xb{i}", name=f"xb{i}")
                 for i in range(3)]

        p1 = top.enter_context(tc.tile_pool(name="p1sb", bufs=2))
        w1 = top.enter_context(tc.tile_pool(name="w1", bufs=1))
        pch = top.enter_context(tc.tile_pool(name="pch", bufs=2))
        epool = top.enter_context(tc.tile_pool(name="epool", bufs=11))
        pnorm = top.enter_context(tc.tile_pool(name="pnorm", bufs=1))

        wq = [w1.tile([128, FP8_FROM * 128], f32r, tag=f"wq{d}", name=f"wq{d}")
              for d in range(ET)]
        wk = [w1.tile([128, FP8_FROM * 128], f32r, tag=f"wk{d}", name=f"wk{d}")
              for d in range(ET)]
        wq8 = [w1.tile([128, 2, W8C], f8, tag=f"wq8{m}", name=f"wq8{m}")
               for m in range(ET // 2)]
        wk8 = [w1.tile([128, 2, W8C], f8, tag=f"wk8{m}", name=f"wk8{m}")
               for m in range(ET // 2)]
        wv = [w1.tile([128, D], bf16, tag=f"wv{d}", name=f"wv{d}")
              for d in range(ET)]

        def qslot(et, cs, qsb):
            return qa8[:, 2 * et + cs, qsb * 512 : qsb * 512 + 512]

        def load_xblk(col0, first=False):
            """All x loads ride the SP DMA queue (the cost model charges a
            transfer to its issuing engine, so compute engines must not carry
            DMAs).  Exception: block 0's big f32r block goes on the still-idle
            Pool queue so it runs parallel with SP's fp8 loads at startup."""
            x8 = p1.tile([128, ET, 512], f8, tag="x8", name="x8")
            nc.sync.dma_start(
                x8[:],
                XT8[:, col0 : col0 + 512].rearrange("(d p) s -> p d s", p=128))
            br = p1.tile([128, ET * 512], f32r, tag="xbr", name="xbr")
            eng = nc.gpsimd if first else nc.sync
            for h in range(2):
                eng.dma_start(
                    br[:, h * 4 * 512 : (h + 1) * 4 * 512]
                    .rearrange("p (d s) -> p d s", d=4),
                    xT[h * 4 * 128 : (h + 1) * 4 * 128, col0 : col0 + 512]
                    .rearrange("(d p) s -> p d s", p=128))
            return br, x8

        def theta_chain(xb, et, w_tiles, w8_tiles, bias_tiles, r_on_act=False):
            """Projection + range reduction; returns a [128, 1024] turn tile
            laid out as (cos-arg | sin-arg) for ONE fused Sin call."""
            br, x8 = xb
            ps = psum.tile([128, 512], f32, tag="proj", name="psf", bufs=3)
            if et < FP8_FROM:
                for d in range(ET):
                    nc.tensor.matmul(ps[:], w_tiles[d][:, et * 128 : (et + 1) * 128],
                                     br[:, d * 512 : (d + 1) * 512],
                                     start=(d == 0), stop=(d == ET - 1))
            else:
                e8 = et - FP8_FROM
                for m in range(ET // 2):
                    nc.tensor.matmul(ps[:],
                                     w8_tiles[m][:, :, e8 * 128 : (e8 + 1) * 128],
                                     x8[:, 2 * m : 2 * m + 2, :],
                                     start=(m == 0), stop=(m == ET // 2 - 1),
                                     perf_mode=PM.DoubleRow)
            r = pch.tile([128, 512], f32, tag="r", name="r")
            if r_on_act:
                nc.scalar.activation(r[:], ps[:], Act.Identity,
                                     scale=sc2[et][:], bias=bias_tiles[et][:])
            else:
                nc.vector.tensor_scalar(r[:], ps[:], sc2[et][:],
                                        bias_tiles[et][:], Alu.mult, Alu.add)
            kk = pch.tile([128, 512], f32, tag="kk", name="kk")
            nc.gpsimd.tensor_scalar(kk[:], r[:], MAGIC, MAGIC, Alu.add, Alu.subtract)
            fg = pch.tile([128, 1024], f32, tag="fg", name="fg")
            f = fg[:, 512:]
            nc.vector.scalar_tensor_tensor(f, kk[:], -1.0, r[:],
                                           Alu.mult, Alu.add)
            nc.vector.add_range_wrap(fg[:, :512], f, 0.25, 0.5, 1.0)
            return fg

        rs_acc = pnorm.tile([2, NS], f32, tag="rsacc")

        def kfeature_unit(xb, kr, et, r_on_act=False):
            fg = theta_chain(xb, et, wk, wk8, bk2, r_on_act=r_on_act)
            cs32 = pch.tile([128, 1024], f32, tag="s32", name="cs32")
            nc.scalar.activation(cs32[:], fg[:], Act.Sin, scale=TWOPI)
            nc.gpsimd.tensor_scalar(kr[:, 2 * et, :], cs32[:, :512],
                                    nac[et][:], None, Alu.add)
            nc.gpsimd.tensor_scalar(kr[:, 2 * et + 1, :], cs32[:, 512:],
                                    nas[et][:], None, Alu.add)

        def qfeature_unit(xb, qsb, et):
            # blocks 0-1 run 16 units with no score chunks: DVE is the
            # startup bottleneck there, so the theta scale+bias runs on ACT
            fg = theta_chain(xb, et, wq, wq8, bq2, r_on_act=True)
            nc.scalar.activation(
                qa8[:, 2 * et : 2 * et + 2, qsb * 512 : qsb * 512 + 512],
                fg[:].rearrange("p (c s) -> p c s", c=2),
                Act.Sin, scale=TWOPI)

        etiles = {}       # blk -> [4 e-tiles]
        krs = {}          # blk -> kres tile

        def score_chunk(blk, loc, ns):
            """One [128 keys x 512 queries] score tile + exp."""
            if blk not in etiles:
                etiles[blk] = [epool.tile([128, NS], bf16, tag="e", name="e")
                               for _ in range(4)]
            kr = krs[blk]
            et_t = etiles[blk][loc]
            sl = slice(ns * 512, ns * 512 + 512)
            ps_sim = psum.tile([128, 512], f32, tag="big",
                               name="ps_sim", bufs=4)
            for j in range(ET):
                nc.tensor.matmul(
                    ps_sim[:],
                    kr[:, 2 * j : 2 * j + 2, loc * 128 : (loc + 1) * 128],
                    qa8[:, 2 * j : 2 * j + 2, ns * 512 : ns * 512 + 512],
                    start=(j == 0), stop=(j == ET - 1),
                    perf_mode=PM.DoubleRow)
            nc.scalar.activation(et_t[:, sl], ps_sim[:], Act.Exp,
                                 scale=INV_SQRT_D)

        def emit_u(blocks, first, last):
            """Rowsums + U += x_blk^T @ E over the given blocks' e-tiles."""
            tiles = [(blk * 4 + loc, etiles[blk][loc])
                     for blk in blocks for loc in range(4)]
            # rowsums first (exps are long done; avoids pacing PE on ACT)
            ps_rs = psum.tile([2, NS], f32, tag="rs", name="ps_rs", bufs=2)
            for gi, (tt, et_t) in enumerate(tiles):
                for ns in range(n_ns):
                    sl = slice(ns * 512, ns * 512 + 512)
                    nc.tensor.matmul(ps_rs[:, sl], ones_col[:], et_t[:, sl],
                                     start=(gi == 0),
                                     stop=(gi == len(tiles) - 1))
            for ns in range(n_ns):
                sl = slice(ns * 512, ns * 512 + 512)
                if first:
                    nc.vector.tensor_copy(rs_acc[:, sl], ps_rs[:, sl])
                else:
                    nc.vector.tensor_tensor(rs_acc[:, sl], ps_rs[:, sl],
                                            rs_acc[:, sl], Alu.add)
            if last:
                # rowsum chain closed: 1/rowsum + broadcast overlap final U
                rec = pnorm.tile([1, NS], bf16, tag="rec")
                with nc.allow_low_precision(
                        reason="1/rowsum broadcast runs bf16; it feeds the "
                               "bf16 bc tile"):
                    nc.vector.reciprocal(rec[:], rs_acc[:1, :])
                bc_t = pnorm.tile([128, NS], bf16, tag="bc")
                for ns in range(n_ns):
                    sl = slice(ns * 512, ns * 512 + 512)
                    ps_bc = psum.tile([128, 512], f32, tag="big", name="ps_bc",
                                      bufs=4)
                    nc.tensor.matmul(ps_bc[:], ones_row[:], rec[:, sl],
                                     start=True, stop=True)
                    nc.vector.tensor_copy(bc_t[:, sl], ps_bc[:])
                bc.append(bc_t)
            for ns in range(n_ns):
                sl = slice(ns * 512, ns * 512 + 512)
                for xt in range(ET):
                    ps_u = psum.tile([128, 512], f32, tag="big", name="ps_u",
                                     bufs=4)
                    for gi, (tt, et_t) in enumerate(tiles):
                        nc.tensor.matmul(
                            ps_u[:],
                            xbres[(tt // 4) % 3][:, tt % 4,
                                                 xt * 128 : (xt + 1) * 128],
                            et_t[:, sl],
                            start=(gi == 0), stop=(gi == len(tiles) - 1))
                    if first:
                        nc.vector.tensor_copy(u_ac[xt][:, sl], ps_u[:])
                    else:
                        nc.vector.tensor_tensor(u_ac[xt][:, sl], ps_u[:],
                                                u_ac[xt][:, sl], Alu.add)
                if last:
                    # this query half of U is complete: project through Wv,
                    # normalize, add the V bias and store right away so the
                    # tail pipelines behind the other half's U chains
                    for dt in range(ET):
                        ps_o = psum.tile([128, 512], f32, tag="big",
                                         name="ps_o", bufs=4)
                        for xt in range(ET):
                            nc.tensor.matmul(
                                ps_o[:],
                                wv[xt][:, dt * 128 : (dt + 1) * 128],
                                u_ac[xt][:, sl],
                                start=(xt == 0), stop=(xt == ET - 1))
                        on = pch.tile([128, 512], f32, tag="r", name="on")
                        nc.vector.tensor_tensor(on[:], ps_o[:], bc[0][:, sl],
                                                Alu.mult)
                        nc.scalar.activation(on[:], on[:], Act.Identity,
                                             bias=bvt[dt][:])
                        nc.sync.dma_start(OT[dt * 128 : (dt + 1) * 128, sl],
                                          on[:])

        def emit_block_units(xb, kr, sblk):
            """K features (+ Q features for the query blocks).  DoubleRow
            e-tiles first: they only need the (small, early) x8 DMA, so the
            PE starts sooner on the first block.  Units form one batch of
            Sin activations; score chunks follow as a batch of Exps, keeping
            ACT table swaps at ~2 per iteration (1.3us each)."""
            if sblk == 0:
                # all DoubleRow units first (each weight queue streams in
                # while the previous group computes), f32r last
                for et in range(FP8_FROM, ET):
                    kfeature_unit(xb, kr, et, r_on_act=True)
                for et in range(FP8_FROM, ET):
                    qfeature_unit(xb, sblk, et)
                for et in range(FP8_FROM):
                    kfeature_unit(xb, kr, et, r_on_act=True)
                for et in range(FP8_FROM):
                    qfeature_unit(xb, sblk, et)
            elif sblk < n_qsb:
                # Q units first: iteration 2's score chunks need the full
                # qa8 half; kr(1) is not read until after units(2)
                for et in range(FP8_FROM, ET):
                    qfeature_unit(xb, sblk, et)
                for et in range(FP8_FROM):
                    qfeature_unit(xb, sblk, et)
                for et in range(FP8_FROM, ET):
                    kfeature_unit(xb, kr, et, r_on_act=True)
                for et in range(FP8_FROM):
                    kfeature_unit(xb, kr, et, r_on_act=True)
            else:
                # f32r units first: their xbr block was prefetched a full
                # iteration ago, and early reads release the xbr buffer for
                # the next prefetch (WAR) sooner.  On U-group iterations DVE
                # carries 16 extra U-accumulate adds, so the theta-psum drain
                # (r) moves to ACT there to keep the psf rotation flowing.
                u_iter = sblk in (3, 5, 7)
                for et in list(range(FP8_FROM)) + list(range(FP8_FROM, ET)):
                    kfeature_unit(xb, kr, et, r_on_act=u_iter)

        # ---- main pipeline over key blocks ----
        bc = []
        # score chunks (blk, loc, ns) emitted AFTER each iteration's feature
        # units: one block behind features (deps a full segment old -> the
        # in-order PE stream never blocks), with the query blocks' own
        # scores starting as soon as qa8's matching half exists
        sched = {0: [],
                 1: [(0, loc, 0) for loc in range(4)],
                 2: [(0, loc, 1) for loc in range(4)]
                    + [(1, loc, ns) for ns in range(2) for loc in range(4)]}
        for i in range(3, n_sblk):
            sched[i] = [(i - 1, loc, ns) for ns in range(2) for loc in range(4)]
        # U groups placed where their e-tiles are at least a chunk-batch old:
        # blocks -> (iteration, before-chunks?) with -1 = post-loop
        ugroups = [((0, 1), 3, True), ((2, 3), 5, True), ((4, 5), 7, True),
                   ((6,), 7, False), ((7,), -1, False)]

        # weights in need-order on SP; wq8 rides the ACT queue (idle for
        # the first ~5us) and block-0 xbr the Pool queue, so three transfer
        # streams drain in parallel at startup.  wv (needed only at the tail)
        # and xb16(0) trail the steady-state prefetches by emission order.
        xbs = {0: load_xblk(0, first=True)}
        for m in range(ET // 2):
            nc.scalar.dma_start(
                wk8[m][:], WK8[2 * m * 128 : (2 * m + 2) * 128, :]
                .rearrange("(j p) e -> p j e", p=128))
        for m in range(ET // 2):
            nc.sync.dma_start(
                wq8[m][:], WQ8[2 * m * 128 : (2 * m + 2) * 128, :]
                .rearrange("(j p) e -> p j e", p=128))
        for d in range(ET):
            nc.sync.dma_start(wk[d][:], WkT[d * 128 : (d + 1) * 128, :])
        for d in range(ET):
            nc.sync.dma_start(wq[d][:], WqT[d * 128 : (d + 1) * 128, :])
        for sblk in range(n_sblk):
            xb = xbs.pop(sblk)
            kr = kpool.tile([128, NSLOT, 512], f8, tag="kr", name=f"kr{sblk}")
            krs[sblk] = kr
            if sblk + 1 < n_sblk and sblk >= 1:
                # prefetch the next block's x at iteration top -- the SP queue
                # starts it a full iteration before the first consumer
                xbs[sblk + 1] = load_xblk((sblk + 1) * 512)
            emit_block_units(xb, kr, sblk)
            if sblk == 0:
                xbs[1] = load_xblk(512)
            if sblk == 1:
                for d in range(ET):
                    nc.sync.dma_start(wv[d][:], WvT[d * 128 : (d + 1) * 128, :])
            for gi, (blocks, at_iter, before) in enumerate(ugroups):
                if at_iter == sblk and before:
                    emit_u(blocks, first=(gi == 0), last=False)
            for c in sched[sblk]:
                score_chunk(*c)
            for gi, (blocks, at_iter, before) in enumerate(ugroups):
                if at_iter == sblk and not before:
                    emit_u(blocks, first=(gi == 0), last=False)
            # issued LAST: emit_u of an older group reads blocks sblk-3/sblk-2
            # from the xbres slot this block's x will reuse (slots rotate
            # mod 3) -- program order must keep those reads first
            nc.sync.dma_start(
                xbres[sblk % 3][:],
                XB16[sblk * 512 : (sblk + 1) * 512, :]
                .rearrange("(k p) d -> p k d", p=128))
        # post-loop: last block's scores + final U group + projection tail
        for c in [(n_sblk - 1, loc, ns) for ns in range(2) for loc in range(4)]:
            score_chunk(*c)
        emit_u(ugroups[-1][0], first=False, last=True)

    nc.compile()
    return nc


def _host_prep(x, Wq, bq, Wk, bk, Wv, bv, phase_bias):
    wavelengths = np.arange(1, D + 1, dtype=np.float32) * np.float32(2.0 * math.pi / D)
    inv_wl = (np.float32(1.0) / (wavelengths + np.float32(1e-8))).astype(np.float32)
    sc2 = (inv_wl / TWOPI).astype(np.float32).reshape(ET, 128)
    bq2 = ((bq * inv_wl + phase_bias) / TWOPI).astype(np.float32).reshape(ET, 128)
    bk2 = ((bk * inv_wl + phase_bias) / TWOPI).astype(np.float32).reshape(ET, 128)
    # K-feature means from the weights: theta_k ~ N(bk*ivl + pb, |wk_row|^2 ivl^2)
    mu = (bk * inv_wl + phase_bias).astype(np.float64)
    var = (np.sum(Wk.astype(np.float64) ** 2, axis=1) * inv_wl.astype(np.float64) ** 2)
    damp = np.exp(-var / 2.0)
    nac = (-(np.cos(mu) * damp)).astype(np.float32).reshape(ET, 128)
    nas = (-(np.sin(mu) * damp)).astype(np.float32).reshape(ET, 128)
    WqTf = np.ascontiguousarray(Wq.T).astype(np.float32)
    WkTf = np.ascontiguousarray(Wk.T).astype(np.float32)
    WqT = np.ascontiguousarray(WqTf[:, : FP8_FROM * 128])
    WkT = np.ascontiguousarray(WkTf[:, : FP8_FROM * 128])
    WQ8 = np.ascontiguousarray(WqTf[:, FP8_FROM * 128 :]).astype(F8NP)
    WK8 = np.ascontiguousarray(WkTf[:, FP8_FROM * 128 :]).astype(F8NP)
    WvT = np.ascontiguousarray(Wv.T).astype(BF16NP)
    xT = [np.ascontiguousarray(x[b].T).astype(np.float32) for b in range(x.shape[0])]
    con = np.stack([sc2, bq2, bk2, bv.reshape(ET, 128).astype(np.float32), nac, nas])
    # [6, ET, 128] -> [128, 6*ET] with column layout (kind, et)
    con = np.ascontiguousarray(con.reshape(6 * ET, 128).T).astype(np.float32)
    return xT, WqT, WkT, WQ8, WK8, WvT, con


def kernel(x, Wq, bq, Wk, bk, Wv, bv, phase_bias, _trace=False):
    from concourse.bass_utils import run_bass_kernel_spmd

    x = np.asarray(x, dtype=np.float32)
    xT, WqT, WkT, WQ8, WK8, WvT, con = _host_prep(
        x, np.asarray(Wq, np.float32), np.asarray(bq, np.float32),
        np.asarray(Wk, np.float32), np.asarray(bk, np.float32),
        np.asarray(Wv, np.float32), np.asarray(bv, np.float32),
        np.asarray(phase_bias, np.float32))

    if "prog" not in _cache:
        _cache["prog"] = _build_program()
    nc = _cache["prog"]

    in_maps = []
    for c in range(NCORES):
        b, qb = c // 4, c % 4
        # rotate the key blocks so this core's query rows come first
        # (scores/rowsum/U are invariant under key permutation)
        xp = np.concatenate(
            [xT[b][:, qb * QBLK : (qb + 1) * QBLK],
             xT[b][:, : qb * QBLK],
             xT[b][:, (qb + 1) * QBLK :]], axis=1)
        xp = np.ascontiguousarray(xp)
        xb16 = np.ascontiguousarray(xp.T).astype(BF16NP)
        in_maps.append({
            "xT": xp,
            "XT8": xp.astype(F8NP),
            "XB16": xb16,
            "WqT": WqT, "WkT": WkT, "WQ8": WQ8, "WK8": WK8, "WvT": WvT,
            "CON": con,
        })
    res = run_bass_kernel_spmd(nc, in_maps, core_ids=list(range(NCORES)),
                               trace=_trace)
    out = np.empty((B, S, D), dtype=np.float32)
    for c in range(NCORES):
        b, qb = c // 4, c % 4
        out[b, qb * QBLK : (qb + 1) * QBLK, :] = res.results[c]["OT"].T
    if _trace:
        kernel.last_exec_time_ns = res.exec_time_ns
        kernel.last_result = res
    return out
